# revision 53
# baseline (speedup 1.0000x reference)
"""Trainium2 Bass kernel for the tree-LSTM decoder (nn_Decoder).

Model (per batch item):
  T=256 sequential LSTM steps with a parent-state gather feeding the input,
  followed per step by two general-attention blocks and an output projection.

Strategy (v4):
  - Data-parallel over batch: B=32 across 8 cores -> 4 items/core.
  - The recurrence runs fully in transposed (feature-on-partition) space:
      gates^T[g, b] = W2^T-chunk (stationary lhsT) x state^T-chunk (moving rhs)
    as [128, 4] outputs: 16 m-chunks x 8 k-chunks + 16 X1 injects per step of
    tiny matmuls (~5 ns each in the PE cost model) instead of 512-col ones,
    and h is produced already transposed (zero transposes per step).
  - Phase A computes X1^T = [Wx^T; bias] @ xe^T directly into a resident
    SBUF tile in gate-chunk-major order (no DRAM roundtrip); bias added via
    a k=1 matmul against the xe ones-row.
  - Parent states come from a gpsimd ap_gather over the SBUF h-history
    (per-column dynamic indices, invalid parents -> a zeroed slot), so there
    is no DRAM h-buffer, no per-step DMA, and no flag blending at all.
  - Gate columns are pre-permuted [i, f, o, g]; per-gate group closes are
    ordered g, i, f, o with activations fired as soon as their gate closes.
  - Phase C (both attentions + output head) batched over all steps/item.
"""

import os
import numpy as np

import concourse.bass as bass
import concourse.bacc as bacc
import concourse.mybir as mybir
import concourse.tile as tile
from concourse.bass_utils import run_bass_kernel_spmd
from concourse.masks import make_identity

F32 = mybir.dt.float32
F32R = mybir.dt.float32r
I16 = mybir.dt.int16
BF16 = mybir.dt.bfloat16
AF = mybir.ActivationFunctionType
AX = mybir.AxisListType

B, H, E = 32, 512, 512
LS = LR = 512
G4 = 4 * H            # 2048
KE = 3 * E + 1        # 1537 (embeddings + ones column for bias)
NCORES = 8
BL = B // NCORES      # 4 local items

TT = int(os.environ.get("KERNEL_T_STEPS", "256"))
PHASES = os.environ.get("KERNEL_PHASES", "abc")
MT = TT * BL          # columns of xeT / X1T, (t, b) item-fast

_BUILT = {}


def _build(nc_cls=bacc.Bacc):
    nc = nc_cls("TRN2")

    # ---------------- I/O ----------------
    din = lambda n, s, d=F32: nc.dram_tensor(n, s, d, kind="ExternalInput")
    xeT = din("xeT", [KE, MT], BF16)                 # embeddings^T (+ones row)
    x1wp = din("x1wp", [KE, G4], BF16)               # [Wx^T; bih+bhh], cols [i,f,o,g]
    w2sp = din("w2sp", [2 * H, G4], F32R)            # [Whh^T; Wp^T], cols [i,f,o,g]
    winsT = din("winsT", [H, H], F32R)
    woutsT = din("woutsT", [2 * H, H], F32R)
    winvT = din("winvT", [H, H], F32R)
    woutvT = din("woutvT", [2 * H, H], F32R)
    walT = din("walT", [3 * H, H], F32R)
    balr = din("balr", [128, 4])               # bal rearranged (hc p) -> p hc
    scT = din("scT", [BL, H, LS], F32R)              # src ctx^T per item [h, l]
    scN = din("scN", [BL, LS, H], F32R)              # src ctx per item [l, h]
    rcT = din("rcT", [BL, H, LR], F32R)
    rcN = din("rcN", [BL, LR, H], F32R)
    mka_s = din("mka_s", [BL, LS], F32R)             # (mask-1)*1e9 rows
    mka_r = din("mka_r", [BL, LR], F32R)
    gidxT = din("gidxT", [4 * BL, TT], mybir.dt.int32)  # gather rows, sentinel 0..15
    f1m = din("f1m", [128, TT, BL])            # flag p==t-1, pre-broadcast [p, t, b]
    f2m = din("f2m", [128, TT, BL])            # flag p==t-2
    f3m = din("f3m", [128, TT, BL])            # flag p==t-3
    id16r = din("id16r", [16, 16], F32R)
    h0Tr = din("h0Tr", [128, 4, BL], F32R)     # h0^T (hc p) b -> p hc b
    c0Tr = din("c0Tr", [128, 4, BL])
    onesr = din("onesr", [1, 128], F32R)

    out_a = nc.dram_tensor("out_a", [BL, TT, H], F32, kind="ExternalOutput")
    out_sc = nc.dram_tensor("out_sc", [BL, TT, LR], F32, kind="ExternalOutput")
    DBG = os.environ.get("KERNEL_DEBUG", "")
    if DBG:
        dbg_x1 = nc.dram_tensor("dbg_x1", [128, 16, MT], F32R, kind="ExternalOutput")
        dbg_h = nc.dram_tensor("dbg_h", [128, TT + 1, 4, BL], F32R, kind="ExternalOutput")

    with tile.TileContext(nc) as tc:
        with (
            tc.tile_pool(name="dram", bufs=1, space="DRAM") as dp,
            tc.tile_pool(name="const", bufs=1) as cp,
        ):
            hbuf = dp.tile([(TT + 1) * 16, 128], F32R)  # 16 sentinel + (t,hc,b) rows
            zrow = cp.tile([16, 128], F32)
            nc.vector.memset(zrow, 0.0)
            nc.gpsimd.dma_start(hbuf[0:16, :], zrow)
            ident = cp.tile([128, 128], F32)
            make_identity(nc, ident)
            identR = cp.tile([128, 128], F32R)
            nc.vector.tensor_copy(identR, ident)
            identB = cp.tile([128, 128], BF16)
            nc.vector.tensor_copy(identB, ident)
            ones_row = cp.tile([1, 128], F32R)
            nc.sync.dma_start(ones_row, onesr[:])
            bal_sb = cp.tile([128, 4], F32)
            nc.sync.dma_start(bal_sb, balr[:])

            # h history: slot s holds h_{s-1}
            hbT = cp.tile([128, TT + 1, 4, BL], F32R)
            c_T = cp.tile([128, 4, BL], F32)
            nc.sync.dma_start(hbT[:, 0, :, :], h0Tr[:])
            nc.sync.dma_start(c_T, c0Tr[:])
            id16_sb = cp.tile([16, 16], F32R)
            nc.sync.dma_start(id16_sb, id16r[:])

            # W2 (stationary operands) + resident X1T; pool closes before C
            pw2_ctx = tc.tile_pool(name="pb_w2", bufs=1)
            pw2 = pw2_ctx.__enter__()
            w2_sb = pw2.tile([128, 8, G4], F32R)
            x1T = pw2.tile([128, 16, MT], BF16)
            gidx_sb = pw2.tile([4 * BL, TT], mybir.dt.int32)
            f1m_sb = pw2.tile([128, TT, BL], F32)
            f2m_sb = pw2.tile([128, TT, BL], F32)
            f3m_sb = pw2.tile([128, TT, BL], F32)

            # ===== Phase A: X1T = [Wx^T; b] @ xeT, 256-col blocks =====
            # Weights live in SBUF (bf16); block 0 runs before the LSTM,
            # later blocks are interleaved into phase B's PE idle windows.
            CB = 256
            NBLK = (MT + CB - 1) // CB
            pa_ctx = [
                tc.tile_pool(name="pa_xe", bufs=2),
                tc.tile_pool(name="pa_w", bufs=1),
                tc.tile_pool(name="pa_ps", bufs=2, space="PSUM"),
            ]
            pxe, pw1, pps = [c.__enter__() for c in pa_ctx]
            x1w_sb = pw1.tile([128, 12, G4], BF16)
            for kc in range(12):
                nc.sync.dma_start(x1w_sb[:, kc, :], x1wp[kc * 128:(kc + 1) * 128, :])
            x1b_sb = pw1.tile([1, G4], BF16)
            nc.sync.dma_start(x1b_sb, x1wp[1536:1537, :])

            def load_xeh(j, queue=None):
                ncol = min(CB, MT - j * CB)
                cs = slice(j * CB, j * CB + ncol)
                xeh = pxe.tile([128, 12, CB], BF16, tag="xeh")
                (queue or nc.sync).dma_start(
                    xeh[:, :, :ncol],
                    xeT[0:1536, cs].rearrange("(a p) c -> p a c", p=128),
                )
                xel = pxe.tile([1, CB], BF16, tag="xel")
                (queue or nc.sync).dma_start(xel[:, :ncol], xeT[1536:1537, cs])
                return xeh, xel

            def emit_a_group(j, mc, xeh, xel):
                ncol = min(CB, MT - j * CB)
                cs = slice(j * CB, j * CB + ncol)
                ps = pps.tile([128, CB], F32, tag="ps")
                for kc in range(12):
                    nc.tensor.matmul(
                        ps[:, :ncol], lhsT=x1w_sb[:, kc, mc * 128:(mc + 1) * 128],
                        rhs=xeh[:, kc, :ncol], start=(kc == 0), stop=False,
                    )
                nc.tensor.matmul(
                    ps[:, :ncol], lhsT=x1b_sb[:, mc * 128:(mc + 1) * 128],
                    rhs=xel[:, :ncol], start=False, stop=True,
                )
                return ps

            nblk_pre = NBLK if 'b' not in PHASES else min(1, NBLK)
            if 'a' in PHASES:
                for j in range(nblk_pre):
                    xeh, xel = load_xeh(j)
                    for mc in range(16):
                        ps = emit_a_group(j, mc, xeh, xel)
                        nc.vector.tensor_copy(
                            x1T[:, mc, j * CB:j * CB + min(CB, MT - j * CB)],
                            ps[:, :min(CB, MT - j * CB)])

            # heavy loads ordered by first use: W2 h-chunks feed step 0,
            # parent chunks and flags feed step 1+
            for kc in range(8):
                nc.sync.dma_start(w2_sb[:, kc, :], w2sp[kc * 128:(kc + 1) * 128, :])
            nc.sync.dma_start(gidx_sb, gidxT[:])
            nc.sync.dma_start(f1m_sb, f1m[:])
            nc.sync.dma_start(f2m_sb, f2m[:])
            nc.sync.dma_start(f3m_sb, f3m[:])

            # schedule for interleaved blocks: step -> (j, mc)
            a_sched = {}
            a_loads = {}
            if 'a' in PHASES and 'b' in PHASES:
                for j in range(nblk_pre, NBLK):
                    a_loads[(j - 1) * 64] = j
                    for mc in range(16):
                        a_sched[(j - 1) * 64 + 2 + 3 * mc] = (j, mc)

            # ================= Phase B: sequential LSTM =================
            with (
                tc.tile_pool(name="pb_par", bufs=3) as ppar,
                tc.tile_pool(name="pb_act", bufs=3) as pact,
                tc.tile_pool(name="pb_dve", bufs=3) as pdve,
                tc.tile_pool(name="pb_gps", bufs=2, space="PSUM") as pgps,
                tc.tile_pool(name="pb_tps", bufs=1, space="PSUM") as ptps,
            ):
                def gather(t):
                    # 16 rows of 128 = parent h^T chunks for (hc, b); only
                    # rows of h_{<=t-4} are referenced (flags cover t-1..t-3)
                    pr = ppar.tile([16, 128], F32R, tag="praw", bufs=4)
                    nc.gpsimd.indirect_dma_start(
                        out=pr, out_offset=None, in_=hbuf[0:max(16, (t - 2) * 16), :],
                        in_offset=bass.IndirectOffsetOnAxis(
                            ap=gidx_sb[:, t:t + 1], axis=0),
                    )
                    return pr

                gat = {}          # step -> gathered [16,128] tile
                sold = {}         # step -> s_old tile (all but the f1 term)
                fb = lambda f, t: f[:, t:t + 1, :].to_broadcast([128, 4, BL])

                def emit_sold(t):
                    # s_old(t) = T(gather(t)) + f2*h_{t-2} + f3*h_{t-3}; every
                    # input is ready one step early, so this runs off the
                    # h->h critical path (all on DVE; Pool head is busy with
                    # the gather SWDGE generation ~1us/step).
                    if t >= TT or t < 2:
                        return
                    pm2 = ppar.tile([128, 4, BL], F32, tag="pm2")
                    nc.vector.tensor_mul(pm2, hbT[:, t - 1, :, :], fb(f2m_sb, t))
                    if t >= 3:
                        pm3 = ppar.tile([128, 4, BL], F32, tag="pm3")
                        nc.vector.tensor_mul(pm3, hbT[:, t - 2, :, :], fb(f3m_sb, t))
                        pp = ppar.tile([128, 4, BL], F32, tag="pp")
                        nc.vector.tensor_add(pp, pm2, pm3)
                    else:
                        pp = pm2
                    if t >= 4:
                        psp = ptps.tile([128, 16], F32R, tag="tpsp")
                        nc.tensor.transpose(psp, gat.pop(t), id16_sb)
                        so = ppar.tile([128, 4, BL], F32R, tag="sold")
                        nc.vector.tensor_add(
                            so, psp.rearrange("p (a b) -> p a b", a=4), pp
                        )
                    else:
                        so = pp
                    sold[t] = so

                a_pend = {}
                pre = {}
                for t in (range(TT) if 'b' in PHASES else []):
                    if t in a_loads:
                        a_pend['x'] = load_xeh(a_loads[t], queue=nc.gpsimd)
                    # fresh part of parT(t): only the f1*h_{t-1} term
                    if t >= 1:
                        parT = ppar.tile([128, 4, BL], F32R, tag="parT")
                        if t == 1:
                            nc.vector.tensor_mul(parT, hbT[:, t, :, :], fb(f1m_sb, t))
                        else:
                            bm1 = pdve.tile([128, 4, BL], F32, tag="bm1")
                            nc.vector.tensor_mul(bm1, hbT[:, t, :, :], fb(f1m_sb, t))
                            nc.vector.tensor_add(parT, sold.pop(t), bm1)
                    emit_sold(t + 1)
                    # gather(t+3) touches only h_{<=t-1} rows; write(t-1)
                    # has completed by mid-step t; the sold DVE ops fill the
                    # c-chain's sif-wait gap instead of delaying bm1(t+1).
                    if 4 <= t + 3 < TT:
                        gat[t + 3] = gather(t + 3)

                    # Three psum tiles (three banks): acts' semaphore waits
                    # are tile-granular, so each gate's activation releases
                    # at its own close instead of the step's last matmul.
                    psG = pgps.tile([128, 4, BL], F32, tag="gG", bufs=1)
                    psIF = pgps.tile([128, 2, 4, BL], F32, tag="gIF")
                    psO = pgps.tile([128, 4, BL], F32, tag="gO", bufs=1)

                    def gps(gt):
                        # psum slice for permuted gate gt (0=i,1=f,2=o,3=g)
                        if gt == 3:
                            return psG
                        if gt == 2:
                            return psO
                        return psIF[:, gt]

                    # inject X1T column block; start=True only on each
                    # tile's first matmul (start zeroes the whole 2KB bank)
                    for mc in range(16):
                        nc.tensor.matmul(
                            gps(mc // 4)[:, mc % 4, :], lhsT=identB,
                            rhs=x1T[:, mc, t * BL:(t + 1) * BL],
                            start=(mc in (0, 8, 12)), stop=False,
                        )
                    # h_{t-1} contributions (ready at step start), i/f first
                    for kc in range(3):
                        for gt in (0, 1, 3, 2):
                            for hc in range(4):
                                mc = gt * 4 + hc
                                nc.tensor.matmul(
                                    gps(gt)[:, hc, :],
                                    lhsT=w2_sb[:, kc, mc * 128:(mc + 1) * 128],
                                    rhs=hbT[:, t, kc, :],
                                    start=False, stop=False,
                                )
                    # per-gate: parent chunks + final h chunk; close i/f
                    # first (the c chain waits on sigmoid(i,f)), then g, o
                    for gt in (0, 1, 3, 2):               # close i, f, g, o
                        if t >= 1:
                            for kc in range(4, 8):
                                for hc in range(4):
                                    mc = gt * 4 + hc
                                    nc.tensor.matmul(
                                        gps(gt)[:, hc, :],
                                        lhsT=w2_sb[:, kc, mc * 128:(mc + 1) * 128],
                                        rhs=parT[:, kc - 4, :],
                                        start=False, stop=False,
                                    )
                        for hc in range(4):
                            mc = gt * 4 + hc
                            nc.tensor.matmul(
                                gps(gt)[:, hc, :],
                                lhsT=w2_sb[:, 3, mc * 128:(mc + 1) * 128],
                                rhs=hbT[:, t, 3, :],
                                start=False,
                                stop=(hc == 3 and gt != 0),
                            )
                        if gt == 1:
                            sif = pact.tile([128, 2, 4, BL], F32, tag="sif")
                            nc.scalar.activation(
                                sif.rearrange("p a b c -> p (a b c)"),
                                psIF.rearrange("p a b c -> p (a b c)"),
                                AF.Sigmoid,
                            )
                        elif gt == 3:
                            tg = pact.tile([128, 4, BL], F32, tag="tg")
                            nc.scalar.activation(tg, psG, AF.Tanh)
                        elif gt == 2:
                            so_ = pact.tile([128, 4, BL], F32, tag="so_")
                            nc.scalar.activation(so_, psO, AF.Sigmoid)
                    if t in a_sched:
                        # hidden phase-A work: runs in the PE idle window
                        # while the activation/c chain produces h_t
                        j, mc = a_sched[t]
                        xeh, xel = a_pend['x']
                        a_pend['c'] = (j, mc, emit_a_group(j, mc, xeh, xel))
                    m1 = pdve.tile([128, 4, BL], F32, tag="m1")
                    nc.vector.tensor_mul(m1, sif[:, 1], c_T)
                    m2 = pdve.tile([128, 4, BL], F32, tag="m2")
                    nc.vector.tensor_mul(m2, sif[:, 0], tg)
                    nc.vector.tensor_add(c_T, m1, m2)
                    tcn = pact.tile([128, 4, BL], F32, tag="tcn")
                    nc.scalar.activation(tcn, c_T, AF.Tanh)
                    nc.vector.tensor_mul(hbT[:, t + 1, :, :], so_, tcn)
                    if 'c' in a_pend:
                        j, mc, aps = a_pend.pop('c')
                        ncol = min(CB, MT - j * CB)
                        nc.vector.tensor_copy(
                            x1T[:, mc, j * CB:j * CB + ncol], aps[:, :ncol])
                    # DRAM rows (t, hc, b) for future gathers, one DMA/step
                    r0 = 16 + t * 16
                    nc.sync.dma_start(
                        hbuf[r0:r0 + 16, :].rearrange("(hc b) p -> p hc b", hc=4),
                        hbT[:, t + 1, :, :],
                    )

            for c_ in reversed(pa_ctx):
                c_.__exit__(None, None, None)
            if DBG:
                nc.sync.dma_start(dbg_x1[:], x1T)
                nc.sync.dma_start(dbg_h[:], hbT)
            pw2_ctx.__exit__(None, None, None)

            # ================= Phase C: attention + output =================
            NMT = TT // 128 if TT >= 128 else 1
            TC = TT // NMT
            with (
                tc.tile_pool(name="pc_w", bufs=1) as pcw,
                tc.tile_pool(name="pc_ctx", bufs=2) as pctx,
                tc.tile_pool(name="pc_q", bufs=2) as pq,
                tc.tile_pool(name="pc_sm", bufs=3) as psm,
                tc.tile_pool(name="pc_ps", bufs=4, space="PSUM") as pcps,
                tc.tile_pool(name="pc_tp", bufs=4, space="PSUM") as pctp,
            ):
                def loadw(apT, kcs, name):
                    t_ = pcw.tile([128, kcs, H], F32R, tag=name)
                    nc.sync.dma_start(
                        t_, apT[:].rearrange("(a p) h -> p a h", p=128)
                    )
                    return t_

                wins_sb = loadw(winsT, 4, "wins")
                wouts_sb = loadw(woutsT, 8, "wouts")
                winv_sb = loadw(winvT, 4, "winv")
                woutv_sb = loadw(woutvT, 8, "woutv")
                wal_sb = loadw(walT, 12, "wal")
                mks_sb = []
                mkr_sb = []
                for bl in range(BL):
                    ts_ = pcw.tile([1, LS], F32R, tag=f"mks{bl}")
                    nc.sync.dma_start(ts_, mka_s[bl:bl + 1, :])
                    mks_sb.append(ts_)
                    tr_ = pcw.tile([1, LR], F32R, tag=f"mkr{bl}")
                    nc.sync.dma_start(tr_, mka_r[bl:bl + 1, :])
                    mkr_sb.append(tr_)

                for bl in (range(BL) if 'c' in PHASES else []):
                    if bl == 0 and 0 in pre:
                        ctxTs, ctxNs, ctxTr, ctxNr = (
                            pre[0], pre[1], pre[2], pre[3])
                    else:
                        ctxTs = pctx.tile([128, 4, LS], F32R, tag="ctxTs")
                        nc.sync.dma_start(
                            ctxTs, scT[bl].rearrange("(a p) l -> p a l", p=128))
                        ctxNs = pctx.tile([128, 4, H], F32R, tag="ctxNs")
                        nc.sync.dma_start(
                            ctxNs, scN[bl].rearrange("(a p) h -> p a h", p=128))
                        ctxTr = pctx.tile([128, 4, LR], F32R, tag="ctxTr", bufs=1)
                        nc.sync.dma_start(
                            ctxTr, rcT[bl].rearrange("(a p) l -> p a l", p=128))
                        ctxNr = pctx.tile([128, 4, H], F32R, tag="ctxNr", bufs=1)
                        nc.sync.dma_start(
                            ctxNr, rcN[bl].rearrange("(a p) h -> p a h", p=128))

                    def hT_read(kc):
                        return hbT[:, 1:TT + 1, kc, bl]

                    def attn(q_read, win_sb, wout_sb, ctxT, ctxN, mk_sb, sc_out):
                        # qpT[h',t] = win^T.T @ qT
                        qpT = pq.tile([128, 4, TT], F32R, tag="qpT", bufs=1)
                        for mh in range(4):
                            ps = pcps.tile([128, TT], F32, tag="cps")
                            for kc in range(4):
                                nc.tensor.matmul(
                                    ps,
                                    lhsT=win_sb[:, kc, mh * 128:(mh + 1) * 128],
                                    rhs=q_read(kc),
                                    start=(kc == 0), stop=(kc == 3),
                                )
                            nc.vector.tensor_copy(qpT[:, mh, :], ps)
                        # scores[t,l] = qpT.T @ ctxT  (+ mask row via ones)
                        alignT = pq.tile([128, 4, TT], F32R, tag="alignT", bufs=1)
                        for mt in range(NMT):
                            ps = pcps.tile([128, LS], F32, tag="cps")
                            for kc in range(4):
                                nc.tensor.matmul(
                                    ps[:TC, :],
                                    lhsT=qpT[:, kc, mt * TC:(mt + 1) * TC],
                                    rhs=ctxT[:, kc, :],
                                    start=(kc == 0), stop=False,
                                )
                            nc.tensor.matmul(
                                ps[:TC, :], lhsT=ones_row[:, :TC], rhs=mk_sb[bl],
                                start=False, stop=True,
                            )
                            # softmax over l (free dim); scores are small
                            # (|s| < ~30) so the max-shift is skipped --
                            # mathematically identical, removes a DVE reduce
                            # from the chain
                            esc = psm.tile([128, LS], F32, tag="esc")
                            rsm = psm.tile([128, 1], F32, tag="rsm")
                            nc.scalar.activation(
                                esc[:TC, :], ps[:TC, :], AF.Exp,
                                accum_out=rsm[:TC, :],
                            )
                            rin = psm.tile([128, 1], F32, tag="rin")
                            nc.vector.reciprocal(rin[:TC, :], rsm[:TC, :])
                            alg = psm.tile([128, LS], F32, tag="alg")
                            nc.vector.tensor_scalar_mul(alg[:TC, :], esc[:TC, :], rin[:TC, :])
                            if sc_out is not None:
                                nc.sync.dma_start(
                                    sc_out[bl, mt * TC:(mt + 1) * TC, :], alg[:TC, :]
                                )
                            for lc in range(4):
                                tp = pctp.tile([128, 128], F32, tag="ctp")
                                nc.tensor.transpose(
                                    tp[:, :TC], alg[:TC, lc * 128:(lc + 1) * 128],
                                    ident[:TC, :TC],
                                )
                                nc.vector.tensor_copy(
                                    alignT[:, lc, mt * TC:(mt + 1) * TC], tp[:, :TC]
                                )
                        # cvecT[h,t] = ctxN.T @ alignT
                        cvT = pq.tile([128, 4, TT], F32R, tag="cvT", bufs=1)
                        for mh in range(4):
                            ps = pcps.tile([128, TT], F32, tag="cps")
                            for kc in range(4):
                                nc.tensor.matmul(
                                    ps,
                                    lhsT=ctxN[:, kc, mh * 128:(mh + 1) * 128],
                                    rhs=alignT[:, kc, :],
                                    start=(kc == 0), stop=(kc == 3),
                                )
                            nc.vector.tensor_copy(cvT[:, mh, :], ps)
                        # outT = tanh(wout^T.T @ [cvec; q])
                        oT = pq.tile([128, 4, TT], F32R, tag="oT")
                        for mh in range(4):
                            ps = pcps.tile([128, TT], F32, tag="cps")
                            for kc in range(8):
                                rhs = cvT[:, kc, :] if kc < 4 else q_read(kc - 4)
                                nc.tensor.matmul(
                                    ps,
                                    lhsT=wout_sb[:, kc, mh * 128:(mh + 1) * 128],
                                    rhs=rhs,
                                    start=(kc == 0), stop=(kc == 7),
                                )
                            nc.scalar.activation(oT[:, mh, :], ps, AF.Tanh)
                        return oT

                    soT = attn(hT_read, wins_sb, wouts_sb, ctxTs, ctxNs, mks_sb, None)
                    voT = attn(
                        lambda kc: soT[:, kc, :], winv_sb, woutv_sb,
                        ctxTr, ctxNr, mkr_sb, out_sc,
                    )

                    # a^T = tanh(wal^T.T @ [h; so; vo] + bal)
                    aT = pq.tile([128, 4, TT], F32, tag="aT", bufs=1)
                    for mh in range(4):
                        ps = pcps.tile([128, TT], F32, tag="cps")
                        for kc in range(12):
                            if kc < 4:
                                rhs = hT_read(kc)
                            elif kc < 8:
                                rhs = soT[:, kc - 4, :]
                            else:
                                rhs = voT[:, kc - 8, :]
                            nc.tensor.matmul(
                                ps,
                                lhsT=wal_sb[:, kc, mh * 128:(mh + 1) * 128],
                                rhs=rhs,
                                start=(kc == 0), stop=(kc == 11),
                            )
                        nc.scalar.activation(
                            aT[:, mh, :], ps, AF.Tanh, bias=bal_sb[:, mh:mh + 1]
                        )
                    # transpose a^T -> [t, h] and write out
                    for mt in range(NMT):
                        am = psm.tile([128, H], F32, tag="am")
                        for mh in range(4):
                            tp = pctp.tile([128, 128], F32, tag="ctp")
                            nc.tensor.transpose(
                                tp[:TC, :], aT[:, mh, mt * TC:(mt + 1) * TC], ident
                            )
                            nc.vector.tensor_copy(
                                am[:TC, mh * 128:(mh + 1) * 128], tp[:TC, :]
                            )
                        nc.sync.dma_start(out_a[bl, mt * TC:(mt + 1) * TC, :], am[:TC, :])

    nc.finalize()
    return nc


GPERM = np.r_[0:1024, 1536:2048, 1024:1536]   # gate cols [i, f, g, o] -> [i, f, o, g]


def _prep_core(inputs, c):
    s = slice(c * BL, (c + 1) * BL)
    f32 = lambda x: np.ascontiguousarray(np.asarray(x), dtype=np.float32)
    i64 = lambda x: np.asarray(x).astype(np.int64)

    nt = i64(inputs["nt"])[s, :TT]
    pr = i64(inputs["prev_rules"])[s, :TT]
    par = i64(inputs["parent_rules"])[s, :TT]
    pidx = i64(inputs["parent_idx"])[s, :TT]
    emb_nt = f32(inputs["emb_nt"])
    emb_rule = f32(inputs["emb_rule"])

    xe = np.concatenate(
        [emb_nt[nt], emb_rule[pr], emb_rule[par]], axis=-1
    )  # [BL, TT, 1536]
    from ml_dtypes import bfloat16
    xeT = np.empty((KE, TT * BL), bfloat16)
    xeT[:1536] = xe.transpose(2, 1, 0).reshape(1536, TT * BL).astype(bfloat16)
    xeT[1536] = 1.0

    tarr = np.arange(TT)[None, :]                      # [1, TT]
    flag1 = (pidx == tarr - 1).astype(np.float32)
    flag2 = (pidx == tarr - 2).astype(np.float32)
    flag3 = (pidx == tarr - 3).astype(np.float32)
    gath = pidx <= tarr - 4                            # [BL, TT]
    hcv = np.arange(4)[:, None, None]
    bv = np.arange(BL)[None, :, None]
    gidx = np.where(
        gath[None, :, :],
        16 + pidx[None, :, :] * 16 + hcv * 4 + bv,
        hcv * 4 + bv,
    ).astype(np.int32).reshape(4 * BL, TT)             # rows (hc*4+b)

    sc = f32(inputs["src_context"])[s]                 # [BL, LS, H]
    rc = f32(inputs["rest_context"])[s]
    smask = f32(inputs["src_mask"])[s]
    rmask = f32(inputs["rest_mask"])[s]
    h0 = f32(inputs["h0"])[s]
    c0 = f32(inputs["c0"])[s]

    return {
        "xeT": xeT,
        "gidxT": gidx,
        "f1m": np.ascontiguousarray(
            np.broadcast_to(flag1.T[None, :, :], (128, TT, BL)).astype(np.float32)),
        "f2m": np.ascontiguousarray(
            np.broadcast_to(flag2.T[None, :, :], (128, TT, BL)).astype(np.float32)),
        "f3m": np.ascontiguousarray(
            np.broadcast_to(flag3.T[None, :, :], (128, TT, BL)).astype(np.float32)),
        "scT": np.ascontiguousarray(sc.transpose(0, 2, 1)),
        "scN": sc,
        "rcT": np.ascontiguousarray(rc.transpose(0, 2, 1)),
        "rcN": rc,
        "mka_s": (smask - 1.0) * 1e9,
        "mka_r": (rmask - 1.0) * 1e9,
        "h0Tr": np.ascontiguousarray(h0.T.reshape(4, 128, BL).transpose(1, 0, 2)),
        "c0Tr": np.ascontiguousarray(c0.T.reshape(4, 128, BL).transpose(1, 0, 2)),
    }


def _prep_shared(inputs):
    f32 = lambda x: np.ascontiguousarray(np.asarray(x), dtype=np.float32)
    Wih = f32(inputs["Wih"])        # [2048, 2048]
    Whh = f32(inputs["Whh"])        # [2048, 512]
    bih = f32(inputs["bih"])
    bhh = f32(inputs["bhh"])
    x1w = np.empty((KE, G4), np.float32)
    x1w[:1536] = Wih[:, :1536].T
    x1w[1536] = bih + bhh
    w2s = np.concatenate([Whh.T, Wih[:, 1536:2048].T], axis=0)  # [1024, 2048]
    bal = f32(inputs["bal"])
    return {
        "onesr": np.ones((1, 128), np.float32),
        "id16r": np.eye(16, dtype=np.float32),
        "x1wp": np.ascontiguousarray(x1w[:, GPERM]).astype(__import__("ml_dtypes").bfloat16),
        "w2sp": np.ascontiguousarray(w2s[:, GPERM]),
        "winsT": np.ascontiguousarray(f32(inputs["Win_s"]).T),
        "woutsT": np.ascontiguousarray(f32(inputs["Wout_s"]).T),
        "winvT": np.ascontiguousarray(f32(inputs["Win_v"]).T),
        "woutvT": np.ascontiguousarray(f32(inputs["Wout_v"]).T),
        "walT": np.ascontiguousarray(f32(inputs["Wal"]).T),
        "balr": np.ascontiguousarray(bal.reshape(4, 128).T),
    }


def kernel(**inputs):
    shared = _prep_shared(inputs)
    in_maps = []
    for c in range(NCORES):
        m = dict(shared)
        m.update(_prep_core(inputs, c))
        in_maps.append(m)

    if "nc" not in _BUILT:
        _BUILT["nc"] = _build()
    nc = _BUILT["nc"]

    trace = os.environ.get("KERNEL_TRACE", "0") == "1"
    res = run_bass_kernel_spmd(
        nc, in_maps, core_ids=list(range(NCORES)), trace=trace
    )
    _BUILT["last_result"] = res
    outs = res.results

    Tfull = np.asarray(inputs["nt"]).shape[1]
    output = np.zeros((B, Tfull, H), np.float32)
    scores = np.zeros((B, Tfull, LR), np.float32)
    for c in range(NCORES):
        output[c * BL:(c + 1) * BL, :TT] = outs[c]["out_a"]
        scores[c * BL:(c + 1) * BL, :TT] = outs[c]["out_sc"]
    return output, scores, scores
